# revision 7
# baseline (speedup 1.0000x reference)
"""CapsuleLayer dynamic-routing kernel for 8 TRN2 NeuronCores (Bass/Tile).

Math restructure (u_hat is never materialized):
    u_hat[b,i,j,d] = sum_k x[b,i,k] W[i,j,k,d]
    s_r[b,(j,d)]   = X[b,(i,k)] @ (c_r odot W)[(i,k),(j,d)]      (matmul, K=(i,k))
    G[(i,k),(j,d)] = X^T @ v_r                                    (matmul, K=b)
    db[i,j]        = sum_{k,d} W[(i,k),(j,d)] * G[(i,k),(j,d)]    (DVE mult+reduce,
                     k-group partition sums via a block-ones matmul)

Sharding: input capsules I=1152 split 8 ways (144 per core). Bias/softmax are
core-local; each routing iteration all-reduces the s partials (256x160 f32)
across the 8 cores; the last iteration reduce-scatters so each core squashes
and emits its own batch shard of v.
"""

import sys

sys.path.insert(0, "/opt/trn_rl_repo")

import numpy as np

import concourse.bacc as bacc
import concourse.bass as bass
import concourse.mybir as mybir
import concourse.tile as tile
from concourse.bass_utils import run_bass_kernel_spmd

F32 = mybir.dt.float32
AF = mybir.ActivationFunctionType
OP = mybir.AluOpType

B, I, DIN, J, D = 256, 1152, 8, 10, 16
NCORES = 8
IL = I // NCORES          # 144 input capsules per core
KI = IL * DIN             # 1152 local contraction length
NT = KI // 128            # 9 K-tiles of 128
JD = J * D                # 160
BL = B // NCORES          # 32 batch rows per core in the final scatter
NUM_ROUTING = 3
EPS = 1e-7


def build():
    nc = bacc.Bacc("TRN2", target_bir_lowering=False, debug=False,
                   num_devices=NCORES)

    xt_d = nc.dram_tensor("xt", [KI, B], F32, kind="ExternalInput")
    x2_d = nc.dram_tensor("x2", [B, KI], F32, kind="ExternalInput")
    w_d = nc.dram_tensor("w", [KI, JD], F32, kind="ExternalInput")
    be_d = nc.dram_tensor("be", [KI, J], F32, kind="ExternalInput")
    ones_d = nc.dram_tensor("ones_blk", [128, 128], F32, kind="ExternalInput")
    out_d = nc.dram_tensor("out", [BL, JD], F32, kind="ExternalOutput")

    groups = [list(range(NCORES))]
    uid = iter(range(10000))

    with tile.TileContext(nc) as tc:
        with (
            tc.tile_pool(name="persist", bufs=1) as pp,
            tc.tile_pool(name="work", bufs=3) as wp,
            tc.tile_pool(name="spsum", bufs=2, space="PSUM") as sp,
            tc.tile_pool(name="gpsum", bufs=4, space="PSUM") as gp,
            tc.tile_pool(name="dbpsum", bufs=2, space="PSUM") as bp,
            tc.tile_pool(name="dram", bufs=1, space="DRAM") as dp,
        ):
            # ---- persistent SBUF arrays ----
            xt_sb = pp.tile([128, NT * B], F32, tag="xt")        # K-tiles side by side
            w_sb = pp.tile([128, NT * JD], F32, tag="w")
            wc_sb = pp.tile([128, NT * JD], F32, tag="wc")
            be_sb = pp.tile([128, NT * J], F32, tag="be")
            c_sb = pp.tile([128, NT * J], F32, tag="c")
            x2a_sb = pp.tile([128, KI], F32, tag="x2a")
            x2b_sb = pp.tile([128, KI], F32, tag="x2b")
            ones_sb = pp.tile([128, 128], F32, tag="ones")
            sf_sb = pp.tile([128, 2 * JD], F32, tag="sf")        # full s, 2 b-tiles
            v_sb = pp.tile([128, 2 * JD], F32, tag="v")

            # ---- input DMAs ----
            for t in range(NT):
                nc.sync.dma_start(out=xt_sb[:, t * B:(t + 1) * B],
                                  in_=xt_d[t * 128:(t + 1) * 128, :])
                nc.sync.dma_start(out=w_sb[:, t * JD:(t + 1) * JD],
                                  in_=w_d[t * 128:(t + 1) * 128, :])
                nc.sync.dma_start(out=be_sb[:, t * J:(t + 1) * J],
                                  in_=be_d[t * 128:(t + 1) * 128, :])
            nc.sync.dma_start(out=x2a_sb[:, :], in_=x2_d[0:128, :])
            nc.sync.dma_start(out=x2b_sb[:, :], in_=x2_d[128:256, :])
            nc.sync.dma_start(out=ones_sb[:, :], in_=ones_d[:, :])

            def softmax_tile(t):
                """c_sb[:, tJ:tJ+J] = softmax(be_sb[:, tJ:tJ+J]) along free dim."""
                bt = be_sb[:, t * J:(t + 1) * J]
                mxn = wp.tile([128, 1], F32, tag="mxn")
                e_t = c_sb[:, t * J:(t + 1) * J]
                z_t = wp.tile([128, 1], F32, tag="z")
                rz_t = wp.tile([128, 1], F32, tag="rz")
                nc.vector.tensor_reduce(out=mxn[:, :], in_=bt, axis=mybir.AxisListType.X,
                                        op=OP.max, negate=True)
                nc.scalar.activation(out=e_t, in_=bt, func=AF.Exp,
                                     bias=mxn[:, :], accum_out=z_t[:, :])
                nc.vector.reciprocal(out=rz_t[:, :], in_=z_t[:, :])
                nc.vector.tensor_scalar_mul(e_t, e_t, rz_t[:, :])

            def squash_cols(s_ap, v_ap, np_, wtag):
                """v = squash(s) over d-segments; s_ap/v_ap are [np_, JD] APs."""
                n = next(uid)
                sq = wp.tile([128, JD], F32, tag=f"sq{wtag}", name=f"sq{n}")[:np_, :]
                s2 = wp.tile([128, J], F32, tag=f"s2{wtag}", name=f"s2_{n}")[:np_, :]
                aux = wp.tile([128, J], F32, tag=f"aux{wtag}", name=f"aux{n}")[:np_, :]
                scl = wp.tile([128, J], F32, tag=f"scl{wtag}", name=f"scl{n}")[:np_, :]
                nc.vector.tensor_tensor(out=sq, in0=s_ap, in1=s_ap, op=OP.mult)
                nc.vector.tensor_reduce(out=s2,
                                        in_=sq.rearrange("p (j d) -> p j d", d=D),
                                        axis=mybir.AxisListType.X, op=OP.add)
                # aux = sqrt(s2+eps) via exp(0.5*ln(s2+eps)) (one ACT table set)
                nc.vector.tensor_scalar_add(aux, s2, EPS)
                nc.scalar.activation(out=aux, in_=aux, func=AF.Ln)
                nc.scalar.activation(out=aux, in_=aux, func=AF.Exp, scale=0.5)
                # scl = s2 / ((1+s2) * sqrt(s2+eps))
                nc.vector.tensor_scalar_add(scl, s2, 1.0)
                nc.vector.tensor_tensor(out=aux, in0=scl, in1=aux, op=OP.mult)
                nc.vector.reciprocal(out=scl, in_=aux)
                nc.vector.tensor_tensor(out=scl, in0=s2, in1=scl, op=OP.mult)
                nc.vector.tensor_tensor(
                    out=v_ap.rearrange("p (j d) -> p j d", d=D),
                    in0=s_ap.rearrange("p (j d) -> p j d", d=D),
                    in1=scl.unsqueeze(2).broadcast_to([np_, J, D]),
                    op=OP.mult)

            for r in range(NUM_ROUTING):
                last = r == NUM_ROUTING - 1
                # -- softmax over output capsules (local i rows, k-expanded) --
                for t in range(NT):
                    softmax_tile(t)
                # -- Wc = c ⊙ W ; s partial = X @ Wc --
                s_ps = [sp.tile([128, JD], F32, tag="s_ps", name=f"s_ps_{r}_{m}")
                        for m in range(2)]
                for t in range(NT):
                    w_t = w_sb[:, t * JD:(t + 1) * JD]
                    wc_t = wc_sb[:, t * JD:(t + 1) * JD]
                    c_t = c_sb[:, t * J:(t + 1) * J]
                    nc.vector.tensor_tensor(
                        out=wc_t.rearrange("p (j d) -> p j d", d=D),
                        in0=w_t.rearrange("p (j d) -> p j d", d=D),
                        in1=c_t.unsqueeze(2).broadcast_to([128, J, D]),
                        op=OP.mult)
                    for m in range(2):
                        nc.tensor.matmul(
                            s_ps[m][:, :],
                            lhsT=xt_sb[:, t * B + m * 128: t * B + (m + 1) * 128],
                            rhs=wc_t,
                            start=(t == 0), stop=(t == NT - 1))
                # -- cross-core reduction of s partials --
                cc_in = dp.tile([B, JD], F32, tag=f"cc_in{r}")
                s_stage = wp.tile([128, 2 * JD], F32, tag="s_stage")
                for m in range(2):
                    nc.scalar.copy(out=s_stage[:, m * JD:(m + 1) * JD],
                                   in_=s_ps[m][:, :])
                    nc.sync.dma_start(out=cc_in[m * 128:(m + 1) * 128, :],
                                      in_=s_stage[:, m * JD:(m + 1) * JD])
                if not last:
                    cc_out = dp.tile([B, JD], F32, tag=f"cc_out{r}")
                    nc.gpsimd.collective_compute(
                        "AllReduce", OP.add, replica_groups=groups,
                        ins=[cc_in[:, :].opt()], outs=[cc_out[:, :].opt()])
                    for m in range(2):
                        nc.sync.dma_start(out=sf_sb[:, m * JD:(m + 1) * JD],
                                          in_=cc_out[m * 128:(m + 1) * 128, :])
                    # -- v = squash(s) --
                    for m in range(2):
                        squash_cols(sf_sb[:, m * JD:(m + 1) * JD],
                                    v_sb[:, m * JD:(m + 1) * JD], 128, "f")
                    # -- G = X^T @ v ; db rows; k-group sum; b += db --
                    for t in range(NT):
                        g_ps = gp.tile([128, JD], F32, tag="g_ps")
                        nc.tensor.matmul(g_ps[:, :],
                                         lhsT=x2a_sb[:, t * 128:(t + 1) * 128],
                                         rhs=v_sb[:, 0:JD],
                                         start=True, stop=False)
                        nc.tensor.matmul(g_ps[:, :],
                                         lhsT=x2b_sb[:, t * 128:(t + 1) * 128],
                                         rhs=v_sb[:, JD:2 * JD],
                                         start=False, stop=True)
                        wg = wp.tile([128, JD], F32, tag="wg")
                        dbr = wp.tile([128, J], F32, tag="dbr")
                        nc.vector.tensor_tensor(
                            out=wg[:, :], in0=g_ps[:, :],
                            in1=w_sb[:, t * JD:(t + 1) * JD], op=OP.mult)
                        nc.vector.tensor_reduce(
                            out=dbr[:, :],
                            in_=wg.rearrange("p (j d) -> p j d", d=D),
                            axis=mybir.AxisListType.X, op=OP.add)
                        db_ps = bp.tile([128, J], F32, tag="db_ps")
                        nc.tensor.matmul(db_ps[:, :], lhsT=ones_sb[:, :],
                                         rhs=dbr[:, :], start=True, stop=True)
                        nc.vector.tensor_tensor(
                            out=be_sb[:, t * J:(t + 1) * J],
                            in0=be_sb[:, t * J:(t + 1) * J],
                            in1=db_ps[:, :], op=OP.add)
                else:
                    rs_out = dp.tile([BL, JD], F32, tag="rs_out")
                    nc.gpsimd.collective_compute(
                        "ReduceScatter", OP.add, replica_groups=groups,
                        ins=[cc_in[:, :].opt()], outs=[rs_out[:, :].opt()])
                    s_loc = wp.tile([128, JD], F32, tag="s_loc",
                                    name="s_loc")[:BL, :]
                    v_loc = wp.tile([128, JD], F32, tag="v_loc",
                                    name="v_loc")[:BL, :]
                    nc.sync.dma_start(out=s_loc, in_=rs_out[:, :])
                    squash_cols(s_loc, v_loc, BL, "l")
                    nc.sync.dma_start(out=out_d[:, :], in_=v_loc)

    nc.compile()
    return nc


_CACHE = {}


def _get_nc():
    if "nc" not in _CACHE:
        _CACHE["nc"] = build()
    return _CACHE["nc"]


def _prep_inputs(inputs, W, bias):
    inputs = np.ascontiguousarray(inputs, dtype=np.float32)
    W4 = np.ascontiguousarray(W, dtype=np.float32).reshape(I, J, DIN, D)
    bias = np.ascontiguousarray(bias, dtype=np.float32)
    ones_blk = np.zeros((128, 128), dtype=np.float32)
    for g in range(16):
        ones_blk[g * 8:(g + 1) * 8, g * 8:(g + 1) * 8] = 1.0
    in_maps = []
    for r in range(NCORES):
        xl = inputs[:, r * IL:(r + 1) * IL, :]                    # [B, IL, DIN]
        xt = np.ascontiguousarray(
            xl.transpose(1, 2, 0).reshape(KI, B))                 # [(i k), b]
        x2 = np.ascontiguousarray(xl.reshape(B, KI))              # [b, (i k)]
        wl = np.ascontiguousarray(
            W4[r * IL:(r + 1) * IL].transpose(0, 2, 1, 3).reshape(KI, JD))
        be = np.ascontiguousarray(
            np.repeat(bias[r * IL:(r + 1) * IL, :], DIN, axis=0))  # [(i k), j]
        in_maps.append({"xt": xt, "x2": x2, "w": wl, "be": be,
                        "ones_blk": ones_blk})
    return in_maps


def run(inputs, W, bias, trace=False, **spmd_kwargs):
    nc = _get_nc()
    in_maps = _prep_inputs(inputs, W, bias)
    res = run_bass_kernel_spmd(nc, in_maps, list(range(NCORES)),
                               trace=trace, **spmd_kwargs)
    v = np.concatenate([res.results[r]["out"] for r in range(NCORES)], axis=0)
    return v.reshape(B, J, D).astype(np.float32), res


def kernel(inputs, W, bias):
    out, _ = run(inputs, W, bias, trace=False)
    return out


# revision 12
# speedup vs baseline: 1.2140x; 1.2140x over previous
"""CapsuleLayer dynamic-routing kernel for 8 TRN2 NeuronCores (Bass/Tile).

Math restructure (u_hat is never materialized):
    u_hat[b,i,j,d] = sum_k x[b,i,k] W[i,j,k,d]
    s_r[b,(j,d)]   = X[b,(i,k)] @ (c_r odot W)[(i,k),(j,d)]      (matmul, K=(i,k))
    G[(i,k),(j,d)] = X^T @ v_r                                    (matmul, K=b)
    db[i,j]        = sum_{k,d} W[(i,k),(j,d)] * G[(i,k),(j,d)]    (DVE mult+reduce,
                     k-group partition sums via a block-ones matmul)

Sharding: input capsules I=1152 split 8 ways (144 per core). Bias/softmax are
core-local; each routing iteration all-reduces the s partials (256x160 f32)
across the 8 cores; the last iteration reduce-scatters so each core squashes
and emits its own batch shard of v.
"""

import sys

sys.path.insert(0, "/opt/trn_rl_repo")

import numpy as np

import concourse.bacc as bacc
import concourse.bass as bass
import concourse.mybir as mybir
import concourse.tile as tile
from concourse.bass_utils import run_bass_kernel_spmd

F32 = mybir.dt.float32
BF16 = mybir.dt.bfloat16
AF = mybir.ActivationFunctionType
OP = mybir.AluOpType

_ONE_ACT_SET = "natural_log_exp_and_others"


def _patch_act_tables():
    """Confine exp/ln (and everything else we use) to a single ACT table set
    so the table-load inserter emits exactly one load instead of thrashing
    between exp_and_others / natural_log (~1.3us per switch)."""
    orig = bacc.get_activation_tables

    def patched(arch):
        t = dict(orig(arch))
        return {k: (v if k == _ONE_ACT_SET else set()) for k, v in t.items()}

    bacc.get_activation_tables = patched

B, I, DIN, J, D = 256, 1152, 8, 10, 16
NCORES = 8
IL = I // NCORES          # 144 input capsules per core
KI = IL * DIN             # 1152 local contraction length
NT = KI // 128            # 9 K-tiles of 128
JD = J * D                # 160
BL = B // NCORES          # 32 batch rows per core in the final scatter
NUM_ROUTING = 3
EPS = 1e-7


def build():
    _patch_act_tables()
    nc = bacc.Bacc("TRN2", target_bir_lowering=False, debug=False,
                   num_devices=NCORES)

    xt_d = nc.dram_tensor("xt", [KI, B], BF16, kind="ExternalInput")
    x2_d = nc.dram_tensor("x2", [B, KI], BF16, kind="ExternalInput")
    w_d = nc.dram_tensor("w", [KI, JD], BF16, kind="ExternalInput")
    be_d = nc.dram_tensor("be", [KI, J], F32, kind="ExternalInput")
    ones_d = nc.dram_tensor("ones_blk", [128, 128], BF16, kind="ExternalInput")
    out_d = nc.dram_tensor("out", [BL, JD], F32, kind="ExternalOutput")

    groups = [list(range(NCORES))]
    uid = iter(range(10000))

    with tile.TileContext(nc) as tc:
        with (
            tc.tile_pool(name="persist", bufs=1) as pp,
            tc.tile_pool(name="work", bufs=3) as wp,
            tc.tile_pool(name="spsum", bufs=2, space="PSUM") as sp,
            tc.tile_pool(name="gpsum", bufs=4, space="PSUM") as gp,
            tc.tile_pool(name="dbpsum", bufs=2, space="PSUM") as bp,
            tc.tile_pool(name="dram", bufs=1, space="DRAM") as dp,
        ):
            # ---- persistent SBUF arrays ----
            xt_sb = pp.tile([128, NT * B], BF16, tag="xt")       # K-tiles side by side
            w_sb = pp.tile([128, NT * JD], BF16, tag="w")
            wc_sb = pp.tile([128, NT * JD], BF16, tag="wc")
            be_sb = pp.tile([128, NT * J], F32, tag="be")
            c_sb = pp.tile([128, NT * J], F32, tag="c")
            x2a_sb = pp.tile([128, KI], BF16, tag="x2a")
            x2b_sb = pp.tile([128, KI], BF16, tag="x2b")
            ones_sb = pp.tile([128, 128], BF16, tag="ones")
            sf_sb = pp.tile([128, 2 * JD], F32, tag="sf")        # full s, 2 b-tiles
            v_sb = pp.tile([128, 2 * JD], BF16, tag="v")

            # ---- input DMAs ----
            for t in range(NT):
                nc.sync.dma_start(out=xt_sb[:, t * B:(t + 1) * B],
                                  in_=xt_d[t * 128:(t + 1) * 128, :])
                nc.sync.dma_start(out=w_sb[:, t * JD:(t + 1) * JD],
                                  in_=w_d[t * 128:(t + 1) * 128, :])
                nc.sync.dma_start(out=be_sb[:, t * J:(t + 1) * J],
                                  in_=be_d[t * 128:(t + 1) * 128, :])
            nc.sync.dma_start(out=x2a_sb[:, :], in_=x2_d[0:128, :])
            nc.sync.dma_start(out=x2b_sb[:, :], in_=x2_d[128:256, :])
            nc.sync.dma_start(out=ones_sb[:, :], in_=ones_d[:, :])

            def softmax_tile(t):
                """c_sb[:, tJ:tJ+J] = softmax(be_sb[:, tJ:tJ+J]) along free dim."""
                bt = be_sb[:, t * J:(t + 1) * J]
                mxn = wp.tile([128, 1], F32, tag="mxn")
                e_t = c_sb[:, t * J:(t + 1) * J]
                z_t = wp.tile([128, 1], F32, tag="z")
                rz_t = wp.tile([128, 1], F32, tag="rz")
                nc.vector.tensor_reduce(out=mxn[:, :], in_=bt, axis=mybir.AxisListType.X,
                                        op=OP.max, negate=True)
                nc.scalar.activation(out=e_t, in_=bt, func=AF.Exp,
                                     bias=mxn[:, :], accum_out=z_t[:, :])
                nc.vector.reciprocal(out=rz_t[:, :], in_=z_t[:, :])
                nc.vector.tensor_scalar_mul(e_t, e_t, rz_t[:, :])

            def squash_cols(s_ap, v_ap, np_, wtag):
                """v = squash(s) over d-segments; s_ap/v_ap are [np_, JD] APs."""
                n = next(uid)
                sq = wp.tile([128, JD], F32, tag=f"sq{wtag}", name=f"sq{n}")[:np_, :]
                s2 = wp.tile([128, J], F32, tag=f"s2{wtag}", name=f"s2_{n}")[:np_, :]
                aux = wp.tile([128, J], F32, tag=f"aux{wtag}", name=f"aux{n}")[:np_, :]
                scl = wp.tile([128, J], F32, tag=f"scl{wtag}", name=f"scl{n}")[:np_, :]
                nc.vector.tensor_tensor(out=sq, in0=s_ap, in1=s_ap, op=OP.mult)
                nc.vector.tensor_reduce(out=s2,
                                        in_=sq.rearrange("p (j d) -> p j d", d=D),
                                        axis=mybir.AxisListType.X, op=OP.add)
                # aux = sqrt(s2+eps) via exp(0.5*ln(s2+eps)) (one ACT table set)
                nc.vector.tensor_scalar_add(aux, s2, EPS)
                nc.scalar.activation(out=aux, in_=aux, func=AF.Ln)
                nc.scalar.activation(out=aux, in_=aux, func=AF.Exp, scale=0.5)
                # scl = s2 / ((1+s2) * sqrt(s2+eps))
                nc.vector.tensor_scalar_add(scl, s2, 1.0)
                nc.vector.tensor_tensor(out=aux, in0=scl, in1=aux, op=OP.mult)
                nc.vector.reciprocal(out=scl, in_=aux)
                nc.vector.tensor_tensor(out=scl, in0=s2, in1=scl, op=OP.mult)
                nc.vector.tensor_tensor(
                    out=v_ap.rearrange("p (j d) -> p j d", d=D),
                    in0=s_ap.rearrange("p (j d) -> p j d", d=D),
                    in1=scl.unsqueeze(2).broadcast_to([np_, J, D]),
                    op=OP.mult)

            for r in range(NUM_ROUTING):
                last = r == NUM_ROUTING - 1
                # -- softmax over output capsules (local i rows, k-expanded) --
                for t in range(NT):
                    softmax_tile(t)
                # -- Wc = c ⊙ W ; s partial = X @ Wc --
                s_ps = [sp.tile([128, JD], F32, tag="s_ps", name=f"s_ps_{r}_{m}")
                        for m in range(2)]
                for t in range(NT):
                    w_t = w_sb[:, t * JD:(t + 1) * JD]
                    wc_t = wc_sb[:, t * JD:(t + 1) * JD]
                    c_t = c_sb[:, t * J:(t + 1) * J]
                    nc.vector.tensor_tensor(
                        out=wc_t.rearrange("p (j d) -> p j d", d=D),
                        in0=w_t.rearrange("p (j d) -> p j d", d=D),
                        in1=c_t.unsqueeze(2).broadcast_to([128, J, D]),
                        op=OP.mult)
                    for m in range(2):
                        nc.tensor.matmul(
                            s_ps[m][:, :],
                            lhsT=xt_sb[:, t * B + m * 128: t * B + (m + 1) * 128],
                            rhs=wc_t,
                            start=(t == 0), stop=(t == NT - 1))
                # -- cross-core reduction of s partials --
                cc_in = dp.tile([B, JD], F32, tag=f"cc_in{r}")
                s_stage = wp.tile([128, 2 * JD], F32, tag="s_stage")
                for m in range(2):
                    nc.scalar.copy(out=s_stage[:, m * JD:(m + 1) * JD],
                                   in_=s_ps[m][:, :])
                    nc.sync.dma_start(out=cc_in[m * 128:(m + 1) * 128, :],
                                      in_=s_stage[:, m * JD:(m + 1) * JD])
                if not last:
                    cc_out = dp.tile([B, JD], F32, tag=f"cc_out{r}")
                    nc.gpsimd.collective_compute(
                        "AllReduce", OP.add, replica_groups=groups,
                        ins=[cc_in[:, :].opt()], outs=[cc_out[:, :].opt()])
                    for m in range(2):
                        nc.sync.dma_start(out=sf_sb[:, m * JD:(m + 1) * JD],
                                          in_=cc_out[m * 128:(m + 1) * 128, :])
                    # -- v = squash(s) --
                    for m in range(2):
                        squash_cols(sf_sb[:, m * JD:(m + 1) * JD],
                                    v_sb[:, m * JD:(m + 1) * JD], 128, "f")
                    # -- G = X^T @ v ; db rows; k-group sum; b += db --
                    for t in range(NT):
                        g_ps = gp.tile([128, JD], F32, tag="g_ps")
                        nc.tensor.matmul(g_ps[:, :],
                                         lhsT=x2a_sb[:, t * 128:(t + 1) * 128],
                                         rhs=v_sb[:, 0:JD],
                                         start=True, stop=False)
                        nc.tensor.matmul(g_ps[:, :],
                                         lhsT=x2b_sb[:, t * 128:(t + 1) * 128],
                                         rhs=v_sb[:, JD:2 * JD],
                                         start=False, stop=True)
                        wg = wp.tile([128, JD], F32, tag="wg")
                        dbr = wp.tile([128, J], BF16, tag="dbr")
                        nc.vector.tensor_tensor(
                            out=wg[:, :], in0=g_ps[:, :],
                            in1=w_sb[:, t * JD:(t + 1) * JD], op=OP.mult)
                        with nc.allow_low_precision(
                                "db rows cast to bf16 for the k-sum matmul"):
                            nc.vector.tensor_reduce(
                                out=dbr[:, :],
                                in_=wg.rearrange("p (j d) -> p j d", d=D),
                                axis=mybir.AxisListType.X, op=OP.add)
                        db_ps = bp.tile([128, J], F32, tag="db_ps")
                        nc.tensor.matmul(db_ps[:, :], lhsT=ones_sb[:, :],
                                         rhs=dbr[:, :], start=True, stop=True)
                        nc.vector.tensor_tensor(
                            out=be_sb[:, t * J:(t + 1) * J],
                            in0=be_sb[:, t * J:(t + 1) * J],
                            in1=db_ps[:, :], op=OP.add)
                else:
                    rs_out = dp.tile([BL, JD], F32, tag="rs_out")
                    nc.gpsimd.collective_compute(
                        "ReduceScatter", OP.add, replica_groups=groups,
                        ins=[cc_in[:, :].opt()], outs=[rs_out[:, :].opt()])
                    s_loc = wp.tile([128, JD], F32, tag="s_loc",
                                    name="s_loc")[:BL, :]
                    v_loc = wp.tile([128, JD], F32, tag="v_loc",
                                    name="v_loc")[:BL, :]
                    nc.sync.dma_start(out=s_loc, in_=rs_out[:, :])
                    squash_cols(s_loc, v_loc, BL, "l")
                    nc.sync.dma_start(out=out_d[:, :], in_=v_loc)

    nc.compile()
    return nc


_CACHE = {}


def _get_nc():
    if "nc" not in _CACHE:
        _CACHE["nc"] = build()
    return _CACHE["nc"]


def _prep_inputs(inputs, W, bias):
    import ml_dtypes
    bf16 = ml_dtypes.bfloat16

    inputs = np.ascontiguousarray(inputs, dtype=np.float32)
    W4 = np.ascontiguousarray(W, dtype=np.float32).reshape(I, J, DIN, D)
    bias = np.ascontiguousarray(bias, dtype=np.float32)
    ones_blk = np.zeros((128, 128), dtype=bf16)
    for g in range(16):
        ones_blk[g * 8:(g + 1) * 8, g * 8:(g + 1) * 8] = 1.0
    in_maps = []
    for r in range(NCORES):
        xl = inputs[:, r * IL:(r + 1) * IL, :]                    # [B, IL, DIN]
        xt = np.ascontiguousarray(
            xl.transpose(1, 2, 0).reshape(KI, B).astype(bf16))    # [(i k), b]
        x2 = np.ascontiguousarray(xl.reshape(B, KI).astype(bf16))  # [b, (i k)]
        wl = np.ascontiguousarray(
            W4[r * IL:(r + 1) * IL].transpose(0, 2, 1, 3)
            .reshape(KI, JD).astype(bf16))
        be = np.ascontiguousarray(
            np.repeat(bias[r * IL:(r + 1) * IL, :], DIN, axis=0))  # [(i k), j]
        in_maps.append({"xt": xt, "x2": x2, "w": wl, "be": be,
                        "ones_blk": ones_blk})
    return in_maps


def run(inputs, W, bias, trace=False, **spmd_kwargs):
    nc = _get_nc()
    in_maps = _prep_inputs(inputs, W, bias)
    res = run_bass_kernel_spmd(nc, in_maps, list(range(NCORES)),
                               trace=trace, **spmd_kwargs)
    v = np.concatenate([res.results[r]["out"] for r in range(NCORES)], axis=0)
    return v.reshape(B, J, D).astype(np.float32), res


def kernel(inputs, W, bias):
    out, _ = run(inputs, W, bias, trace=False)
    return out
